# revision 7
# baseline (speedup 1.0000x reference)
"""Multi-branch BatchNorm2d (16 branches sharing one batch-stat reduction).

Computation (reference):
    mean/var over (B,H,W) per channel of x[32,64,32,32], then for each of
    N=16 branches: out[:, n*64:(n+1)*64] = gamma[n,c]*xhat + beta[n,c],
    giving out[32, 1024, 32, 32].

Strategy (8 NeuronCores, data-parallel over batch):
  - Each core takes B/8 = 4 batches of x (1 MiB), laid out in SBUF as
    [128, 2048] with partition p = c*2 + h0 (h0 = which half of H) and
    free dim (b_local, h1, w).
  - Per-partition partial sums S and sum-of-squares Q via bn_stats/bn_aggr,
    then a 1 KB AllReduce(add) over the 8 cores combines them.
  - mean = S/32768, inv = rsqrt(var+eps) computed per channel (replicated
    over the h0 pair), folded with gamma/beta into per-(branch,channel)
    scale A = gamma*inv and bias B = beta - mean*A.
  - 16 fused tensor_scalar ops (out = x*A + B) + 16 x 1 MiB DMA stores.
    Output traffic (16 MiB/core) is the roofline for this memory-bound
    problem; input x is read exactly once across the chip.
"""

import numpy as np

import concourse.bacc as bacc
import concourse.bass as bass
import concourse.tile as tile
from concourse import mybir
from concourse.bass_utils import run_bass_kernel_spmd

B, C, H, W = 32, 64, 32, 32
N = 16
NCORES = 8
BL = B // NCORES           # 4 batches per core
H2 = H // 2                # 16
FD = BL * H2 * W           # 2048 free elems per partition
NTOT = float(B * H * W)    # 32768 elements reduced per channel
EPS = 1e-5
F32 = mybir.dt.float32

OUT_DMA_GROUP = 1          # branches per output DMA

_NC_CACHE = {}


def _build():
    # Bacc (not raw Bass): its generate_event_semaphores pass legalizes
    # instructions down to <=1 sync-wait each (walrus TS encodings cannot
    # carry more).
    nc = bacc.Bacc("TRN2", num_devices=NCORES, target_bir_lowering=False,
                   debug=False)
    x = nc.dram_tensor("x", [BL, C, H, W], F32, kind="ExternalInput")
    g128 = nc.dram_tensor("g128", [2 * C, N], F32, kind="ExternalInput")
    b128 = nc.dram_tensor("b128", [2 * C, N], F32, kind="ExternalInput")
    out = nc.dram_tensor("out", [BL, N * C, H, W], F32, kind="ExternalOutput")

    # [128, 4, 512]: partition (c h0), free (b, h1*w)
    x_re = x.ap().rearrange("b c (h0 h1) w -> (c h0) b (h1 w)", h0=2)
    # [16, 128, 4, 512]
    out_re = out.ap().rearrange("b (n c) (h0 h1) w -> n (c h0) b (h1 w)", n=N, h0=2)

    with tile.TileContext(nc) as tc:
        with (
            tc.tile_pool(name="xin", bufs=1) as xin,
            tc.tile_pool(name="consts", bufs=1) as consts,
            tc.tile_pool(name="small", bufs=1) as small,
            tc.tile_pool(name="outs", bufs=6) as outs,
            tc.tile_pool(name="dram", bufs=1, space="DRAM") as dram,
        ):
            # Per-(c,h0) gamma/beta, pre-transposed on host: [128, 16].
            g_sb = consts.tile([2 * C, N], F32)
            b_sb = consts.tile([2 * C, N], F32)
            nc.gpsimd.dma_start(out=g_sb, in_=g128.ap())
            nc.gpsimd.dma_start(out=b_sb, in_=b128.ap())

            x_sb = xin.tile([2 * C, BL, H2 * W], F32)
            nc.sync.dma_start(out=x_sb, in_=x_re)
            x_flat = x_sb.rearrange("p b f -> p (b f)")  # [128, 2048]

            # Local stats -> additive partials (S, Q) per partition.
            nchunk = FD // 512
            st = small.tile([128, nchunk, 6], F32)
            for k in range(nchunk):
                nc.vector.bn_stats(out=st[:, k, :], in_=x_flat[:, k * 512:(k + 1) * 512])
            mv = small.tile([128, 2], F32)
            nc.vector.bn_aggr(out=mv, in_=st)

            part = small.tile([128, 2], F32)
            msq = small.tile([128, 1], F32)
            nc.vector.tensor_mul(out=msq, in0=mv[:, 0:1], in1=mv[:, 0:1])
            nc.vector.tensor_scalar_mul(out=part[:, 0:1], in0=mv[:, 0:1], scalar1=float(FD))
            ex2 = small.tile([128, 1], F32)
            nc.vector.tensor_add(out=ex2, in0=mv[:, 1:2], in1=msq)
            nc.vector.tensor_scalar_mul(out=part[:, 1:2], in0=ex2, scalar1=float(FD))

            # 1 KB AllReduce across the 8 cores through DRAM bounce buffers.
            ar_in = dram.tile([128, 2], F32)
            ar_out = dram.tile([128, 2], F32)
            nc.sync.dma_start(out=ar_in, in_=part)
            nc.gpsimd.collective_compute(
                "AllReduce",
                mybir.AluOpType.add,
                replica_groups=[list(range(NCORES))],
                ins=[ar_in.opt()],
                outs=[ar_out.opt()],
            )

            # Read back pair-expanded: partition p=(c,h0) gets [S0,Q0,S1,Q1] of c.
            ar_re = ar_out[:, :].rearrange("(c h0) s -> c (h0 s)", h0=2)
            ar_dup = bass.AP(
                tensor=ar_re.tensor, offset=ar_re.offset,
                ap=[ar_re.ap[0], [0, 2], ar_re.ap[1]],
            )
            sq = small.tile([128, 2, 2], F32)
            nc.sync.dma_start(out=sq, in_=ar_dup)
            stt = small.tile([128, 2], F32)  # (S_tot, Q_tot)
            nc.vector.tensor_add(out=stt, in0=sq[:, 0, :], in1=sq[:, 1, :])

            mean = small.tile([128, 1], F32)
            nc.vector.tensor_scalar_mul(out=mean, in0=stt[:, 0:1], scalar1=1.0 / NTOT)
            ex2t = small.tile([128, 1], F32)
            nc.vector.tensor_scalar_mul(out=ex2t, in0=stt[:, 1:2], scalar1=1.0 / NTOT)
            msq2 = small.tile([128, 1], F32)
            nc.vector.tensor_mul(out=msq2, in0=mean, in1=mean)
            var = small.tile([128, 1], F32)
            nc.vector.tensor_sub(out=var, in0=ex2t, in1=msq2)
            sbuf_eps = small.tile([128, 1], F32)
            nc.vector.memset(sbuf_eps, EPS)
            sd = small.tile([128, 1], F32)
            nc.scalar.activation(out=sd, in_=var,
                                 func=mybir.ActivationFunctionType.Sqrt,
                                 bias=sbuf_eps[:, :])
            inv = small.tile([128, 1], F32)
            nc.vector.reciprocal(out=inv, in_=sd)

            # A = gamma*inv ; Bc = beta - mean*A. On gpsimd: it loaded
            # g_sb/b_sb (same-engine deps), so only `inv`/`mean` (DVE) need
            # one cross-engine wait — keeps every instruction at <=1 wait
            # (TensorScalarPtr encodings have very few sync-wait slots).
            a_sb = consts.tile([128, N], F32)
            nc.gpsimd.tensor_scalar_mul(out=a_sb, in0=g_sb, scalar1=inv)
            ma = consts.tile([128, N], F32)
            nc.gpsimd.tensor_scalar_mul(out=ma, in0=a_sb, scalar1=mean)
            bc_sb = consts.tile([128, N], F32)
            nc.gpsimd.tensor_sub(out=bc_sb, in0=b_sb, in1=ma)

            # Main loop: one fused multiply-add + store per branch group.
            for n0 in range(0, N, OUT_DMA_GROUP):
                grp = OUT_DMA_GROUP
                o = outs.tile([128, grp, FD], F32)
                for j in range(grp):
                    n = n0 + j
                    nc.vector.tensor_scalar(
                        out=o[:, j, :], in0=x_flat,
                        scalar1=a_sb[:, n:n + 1], scalar2=bc_sb[:, n:n + 1],
                        op0=mybir.AluOpType.mult, op1=mybir.AluOpType.add,
                    )
                if grp == 1:
                    nc.sync.dma_start(out=out_re[n0], in_=o[:, 0, :])
                else:
                    # DRAM side iterates partition-major then branch:
                    # [[512,128],[65536,grp],[1048576,4],[1,512]]
                    dst = bass.AP(
                        tensor=out_re.tensor,
                        offset=out_re.offset + n0 * out_re.ap[0][0],
                        ap=[out_re.ap[1], [out_re.ap[0][0], grp]] + out_re.ap[2:],
                    )
                    nc.sync.dma_start(out=dst, in_=o)
    # Run Bacc's compile pipeline (event-sem legalization, register
    # allocation); the PJRT execute path serializes without finalizing.
    nc.finalize()
    return nc


def _get_nc():
    if "nc" not in _NC_CACHE:
        _NC_CACHE["nc"] = _build()
    return _NC_CACHE["nc"]


def _run(inputs, **kwargs):
    x = np.ascontiguousarray(np.asarray(inputs["x"], dtype=np.float32))
    gamma = np.asarray(inputs["gamma"], dtype=np.float32)
    beta = np.asarray(inputs["beta"], dtype=np.float32)
    g128 = np.ascontiguousarray(np.repeat(gamma.T, 2, axis=0))  # [128, 16]
    b128 = np.ascontiguousarray(np.repeat(beta.T, 2, axis=0))
    in_maps = [
        {"x": np.ascontiguousarray(x[i * BL:(i + 1) * BL]),
         "g128": g128, "b128": b128}
        for i in range(NCORES)
    ]
    nc = _get_nc()
    res = run_bass_kernel_spmd(nc, in_maps, core_ids=list(range(NCORES)), **kwargs)
    full = np.concatenate([r["out"] for r in res.results], axis=0)
    return full, res


def kernel(**inputs):
    full, _ = _run(inputs)
    return full


# revision 8
# speedup vs baseline: 1.3788x; 1.3788x over previous
"""Multi-branch BatchNorm2d (16 branches sharing one batch-stat reduction).

Computation (reference):
    mean/var over (B,H,W) per channel of x[32,64,32,32], then for each of
    N=16 branches: out[:, n*64:(n+1)*64] = gamma[n,c]*xhat + beta[n,c],
    giving out[32, 1024, 32, 32].

Strategy (8 NeuronCores, branch-parallel, no collectives):
  - x is replicated: every core reads the full 8 MiB x and computes the
    (B,H,W) mean/var locally. A 1 KB all-reduce would instead allow a
    batch-sharded read (1 MiB/core), but the ncfw collective measures
    70-80 us/call on this setup - far more than the 20 us of extra read.
    With no cross-core dependency, each core's span is independent of
    dispatch stagger.
  - SBUF layout [128, 8, 2048]: partition p = c*2 + h0 (h0 = H half),
    free (batch-chunk of 4, (b, h1, w)). Per-chunk bn_stats pipelines
    behind the chunk DMA loads; one bn_aggr combines 32 stat records.
  - The (c,0)/(c,1) partition pair is summed via a tiny DRAM bounce, then
    mean = S/32768 and inv = rsqrt(var+eps) are folded with gamma/beta
    into per-(branch,channel) scale A = gamma*inv, bias B = beta - mean*A.
  - Each core computes N/8 = 2 branches: 16 fused tensor_scalar ops
    (out = x*A + B) + 16 x 1 MiB DMA stores = 16 MiB of output writes per
    core, the HBM roofline for this memory-bound problem.
"""

import numpy as np

import concourse.bacc as bacc
import concourse.bass as bass
import concourse.tile as tile
from concourse import mybir
from concourse.bass_utils import run_bass_kernel_spmd

B, C, H, W = 32, 64, 32, 32
N = 16
NCORES = 8
NL = N // NCORES           # 2 branches per core
NCHUNK = 8                 # batch chunks for load/stats pipelining
BCH = B // NCHUNK          # 4 batches per chunk
H2 = H // 2                # 16
FD = BCH * H2 * W          # 2048 free elems per partition per chunk
NTOT = float(B * H * W)    # 32768 elements reduced per channel
EPS = 1e-5
F32 = mybir.dt.float32

_NC_CACHE = {}


def _build():
    # Bacc (not raw Bass): its generate_event_semaphores pass legalizes
    # instructions down to <=1 sync-wait each (walrus TS encodings cannot
    # carry more).
    nc = bacc.Bacc("TRN2", num_devices=NCORES, target_bir_lowering=False,
                   debug=False)
    x = nc.dram_tensor("x", [B, C, H, W], F32, kind="ExternalInput")
    gn = nc.dram_tensor("gn", [2 * C, NL], F32, kind="ExternalInput")
    bn = nc.dram_tensor("bn", [2 * C, NL], F32, kind="ExternalInput")
    out = nc.dram_tensor("out", [B, NL * C, H, W], F32, kind="ExternalOutput")

    # [128, 8, 4, 512]: partition (c h0), free (chunk, b, h1*w)
    x_re = x.ap().rearrange("(cb b) c (h0 h1) w -> (c h0) cb b (h1 w)",
                            cb=NCHUNK, h0=2)
    # [2, 128, 8, 4, 512]
    out_re = out.ap().rearrange(
        "(cb b) (n c) (h0 h1) w -> n (c h0) cb b (h1 w)",
        cb=NCHUNK, n=NL, h0=2)

    with tile.TileContext(nc) as tc:
        with (
            tc.tile_pool(name="xin", bufs=1) as xin,
            tc.tile_pool(name="consts", bufs=1) as consts,
            tc.tile_pool(name="small", bufs=1) as small,
            tc.tile_pool(name="outs", bufs=6) as outs,
            tc.tile_pool(name="dram", bufs=1, space="DRAM") as dram,
        ):
            # Per-(c,h0) gamma/beta for this core's branches, pre-transposed
            # on host: [128, 2].
            g_sb = consts.tile([2 * C, NL], F32)
            b_sb = consts.tile([2 * C, NL], F32)
            nc.gpsimd.dma_start(out=g_sb, in_=gn.ap())
            nc.gpsimd.dma_start(out=b_sb, in_=bn.ap())

            # Full x, loaded in batch chunks so stats pipeline behind DMA.
            x_sb = xin.tile([2 * C, NCHUNK, BCH * H2 * W], F32)
            st = small.tile([128, NCHUNK, FD // 512, 6], F32)
            for cb in range(NCHUNK):
                nc.sync.dma_start(out=x_sb[:, cb, :], in_=x_re[:, cb, :, :])
                for k in range(FD // 512):
                    nc.vector.bn_stats(
                        out=st[:, cb, k, :],
                        in_=x_sb[:, cb, k * 512:(k + 1) * 512])
            mv = small.tile([128, 2], F32)
            nc.vector.bn_aggr(out=mv, in_=st)

            # Additive partials (S, Q) per partition (still per H-half).
            fdl = float(NCHUNK * FD)
            part = small.tile([128, 2], F32)
            msq = small.tile([128, 1], F32)
            nc.vector.tensor_mul(out=msq, in0=mv[:, 0:1], in1=mv[:, 0:1])
            nc.vector.tensor_scalar_mul(out=part[:, 0:1], in0=mv[:, 0:1],
                                        scalar1=fdl)
            ex2 = small.tile([128, 1], F32)
            nc.vector.tensor_add(out=ex2, in0=mv[:, 1:2], in1=msq)
            nc.vector.tensor_scalar_mul(out=part[:, 1:2], in0=ex2, scalar1=fdl)

            # Combine the (c,0)/(c,1) partition pair through a DRAM bounce:
            # partition p=(c,h0) reads back [S0,Q0,S1,Q1] of its channel.
            sc = dram.tile([128, 2], F32)
            nc.sync.dma_start(out=sc, in_=part)
            sc_re = sc[:, :].rearrange("(c h0) s -> c (h0 s)", h0=2)
            sc_dup = bass.AP(
                tensor=sc_re.tensor, offset=sc_re.offset,
                ap=[sc_re.ap[0], [0, 2], sc_re.ap[1]],
            )
            sq = small.tile([128, 2, 2], F32)
            nc.sync.dma_start(out=sq, in_=sc_dup)
            stt = small.tile([128, 2], F32)  # (S_tot, Q_tot)
            nc.vector.tensor_add(out=stt, in0=sq[:, 0, :], in1=sq[:, 1, :])

            mean = small.tile([128, 1], F32)
            nc.vector.tensor_scalar_mul(out=mean, in0=stt[:, 0:1],
                                        scalar1=1.0 / NTOT)
            ex2t = small.tile([128, 1], F32)
            nc.vector.tensor_scalar_mul(out=ex2t, in0=stt[:, 1:2],
                                        scalar1=1.0 / NTOT)
            msq2 = small.tile([128, 1], F32)
            nc.vector.tensor_mul(out=msq2, in0=mean, in1=mean)
            var = small.tile([128, 1], F32)
            nc.vector.tensor_sub(out=var, in0=ex2t, in1=msq2)
            sbuf_eps = small.tile([128, 1], F32)
            nc.vector.memset(sbuf_eps, EPS)
            sd = small.tile([128, 1], F32)
            nc.scalar.activation(out=sd, in_=var,
                                 func=mybir.ActivationFunctionType.Sqrt,
                                 bias=sbuf_eps[:, :])
            inv = small.tile([128, 1], F32)
            nc.vector.reciprocal(out=inv, in_=sd)

            # A = gamma*inv ; Bc = beta - mean*A. On gpsimd: it loaded
            # g_sb/b_sb (same-engine deps), so only `inv`/`mean` (DVE) need
            # one cross-engine wait - keeps every instruction at <=1 wait
            # (TensorScalarPtr encodings have very few sync-wait slots).
            a_sb = consts.tile([128, NL], F32)
            nc.gpsimd.tensor_scalar_mul(out=a_sb, in0=g_sb, scalar1=inv)
            ma = consts.tile([128, NL], F32)
            nc.gpsimd.tensor_scalar_mul(out=ma, in0=a_sb, scalar1=mean)
            bc_sb = consts.tile([128, NL], F32)
            nc.gpsimd.tensor_sub(out=bc_sb, in0=b_sb, in1=ma)

            # Main loop: fused multiply-add + 1 MiB store per (branch, chunk).
            for j in range(NL):
                for cb in range(NCHUNK):
                    o = outs.tile([128, FD], F32)
                    nc.vector.tensor_scalar(
                        out=o, in0=x_sb[:, cb, :],
                        scalar1=a_sb[:, j:j + 1], scalar2=bc_sb[:, j:j + 1],
                        op0=mybir.AluOpType.mult, op1=mybir.AluOpType.add,
                    )
                    nc.sync.dma_start(out=out_re[j][:, cb, :, :], in_=o)
    # Run Bacc's compile pipeline (event-sem legalization, register
    # allocation); the PJRT execute path serializes without finalizing.
    nc.finalize()
    return nc


def _get_nc():
    if "nc" not in _NC_CACHE:
        _NC_CACHE["nc"] = _build()
    return _NC_CACHE["nc"]


def _run(inputs, **kwargs):
    x = np.ascontiguousarray(np.asarray(inputs["x"], dtype=np.float32))
    gamma = np.asarray(inputs["gamma"], dtype=np.float32)
    beta = np.asarray(inputs["beta"], dtype=np.float32)
    g128 = np.ascontiguousarray(np.repeat(gamma.T, 2, axis=0))  # [128, 16]
    b128 = np.ascontiguousarray(np.repeat(beta.T, 2, axis=0))
    in_maps = [
        {"x": x,
         "gn": np.ascontiguousarray(g128[:, i * NL:(i + 1) * NL]),
         "bn": np.ascontiguousarray(b128[:, i * NL:(i + 1) * NL])}
        for i in range(NCORES)
    ]
    nc = _get_nc()
    res = run_bass_kernel_spmd(nc, in_maps, core_ids=list(range(NCORES)), **kwargs)
    # Core i computed branches [i*NL, (i+1)*NL) -> channel block of NL*C.
    full = np.concatenate([r["out"] for r in res.results], axis=1)
    return full, res


def kernel(**inputs):
    full, _ = _run(inputs)
    return full


# revision 9
# speedup vs baseline: 1.5112x; 1.0960x over previous
"""Multi-branch BatchNorm2d (16 branches sharing one batch-stat reduction).

Computation (reference):
    mean/var over (B,H,W) per channel of x[32,64,32,32], then for each of
    N=16 branches: out[:, n*64:(n+1)*64] = gamma[n,c]*xhat + beta[n,c],
    giving out[32, 1024, 32, 32].

Strategy (8 NeuronCores, branch-parallel, no collectives):
  - x is replicated: every core reads the full 8 MiB x and computes the
    (B,H,W) mean/var locally. A 1 KB all-reduce would instead allow a
    batch-sharded read (1 MiB/core), but the ncfw collective measures
    70-80 us/call on this setup - far more than the 20 us of extra read.
    With no cross-core dependency, each core's span is independent of
    dispatch stagger.
  - SBUF layout [128, 32, 512]: partition p = c*2 + h0 (h0 = H half),
    free (b, (h1, w)). x is loaded in decreasing batch chunks so the
    per-batch bn_stats pipeline drains right behind the last DMA.
  - The (c,0)/(c,1) partition pair is summed via two small SBUF-to-SBUF
    DMAs (no HBM bounce), then mean = S/32768, inv = rsqrt(var+eps) are
    folded with gamma/beta into per-(branch,channel) scale A = gamma*inv,
    bias B = beta - mean*A.
  - Each core computes N/8 = 2 branches: 16 fused tensor_scalar ops
    (out = x*A + B) + 16 x 1 MiB DMA stores = 16 MiB of output writes per
    core, the HBM roofline for this memory-bound problem.
"""

import numpy as np

import concourse.bacc as bacc
import concourse.bass as bass
import concourse.tile as tile
from concourse import mybir
from concourse.bass_utils import run_bass_kernel_spmd

B, C, H, W = 32, 64, 32, 32
N = 16
NCORES = 8
NL = N // NCORES           # 2 branches per core
H2 = H // 2                # 16
FB = H2 * W                # 512 free elems per batch per partition
NTOT = float(B * H * W)    # 32768 elements reduced per channel
EPS = 1e-5
F32 = mybir.dt.float32

# Load chunks (in batches): big first so the read stream saturates early,
# small last so the stats tail behind the final DMA is short.
CHUNKS = [6, 6, 5, 5, 4, 3, 2, 1]
assert sum(CHUNKS) == B

WG = 4                     # batches per write group

_NC_CACHE = {}


def _build():
    # Bacc (not raw Bass): its generate_event_semaphores pass legalizes
    # instructions down to <=1 sync-wait each (walrus TS encodings cannot
    # carry more).
    nc = bacc.Bacc("TRN2", num_devices=NCORES, target_bir_lowering=False,
                   debug=False)
    x = nc.dram_tensor("x", [B, C, H, W], F32, kind="ExternalInput")
    gn = nc.dram_tensor("gn", [2 * C, NL], F32, kind="ExternalInput")
    bn = nc.dram_tensor("bn", [2 * C, NL], F32, kind="ExternalInput")
    out = nc.dram_tensor("out", [B, NL * C, H, W], F32, kind="ExternalOutput")

    # [128, 32, 512]: partition (c h0), free (b, h1*w)
    x_re = x.ap().rearrange("b c (h0 h1) w -> (c h0) b (h1 w)", h0=2)
    # [2, 128, 32, 512]
    out_re = out.ap().rearrange("b (n c) (h0 h1) w -> n (c h0) b (h1 w)",
                                n=NL, h0=2)

    with tile.TileContext(nc) as tc:
        with (
            tc.tile_pool(name="xin", bufs=1) as xin,
            tc.tile_pool(name="consts", bufs=1) as consts,
            tc.tile_pool(name="small", bufs=1) as small,
            tc.tile_pool(name="outs", bufs=6) as outs,
        ):
            # Per-(c,h0) gamma/beta for this core's branches, pre-transposed
            # on host: [128, 2].
            g_sb = consts.tile([2 * C, NL], F32)
            b_sb = consts.tile([2 * C, NL], F32)
            nc.gpsimd.dma_start(out=g_sb, in_=gn.ap())
            nc.gpsimd.dma_start(out=b_sb, in_=bn.ap())

            # Full x, loaded in batch chunks; per-batch bn_stats pipeline
            # behind each chunk's DMA.
            x_sb = xin.tile([2 * C, B, FB], F32)
            st = small.tile([128, B, 6], F32)
            b0 = 0
            for nb in CHUNKS:
                nc.sync.dma_start(out=x_sb[:, b0:b0 + nb, :],
                                  in_=x_re[:, b0:b0 + nb, :])
                for b in range(b0, b0 + nb):
                    nc.vector.bn_stats(out=st[:, b, :], in_=x_sb[:, b, :])
                b0 += nb
            mv = small.tile([128, 2], F32)
            nc.vector.bn_aggr(out=mv, in_=st)

            # Additive partials (S, Q) per partition (still per H-half).
            part = small.tile([128, 2], F32)
            msq = small.tile([128, 1], F32)
            nc.vector.tensor_mul(out=msq, in0=mv[:, 0:1], in1=mv[:, 0:1])
            nc.vector.tensor_scalar_mul(out=part[:, 0:1], in0=mv[:, 0:1],
                                        scalar1=NTOT / 2.0)
            ex2 = small.tile([128, 1], F32)
            nc.vector.tensor_add(out=ex2, in0=mv[:, 1:2], in1=msq)
            nc.vector.tensor_scalar_mul(out=part[:, 1:2], in0=ex2,
                                        scalar1=NTOT / 2.0)

            # Pair-combine via SBUF-to-SBUF DMA: partition c of sq64 gets
            # [S0, Q0, S1, Q1] of channel c (reading partitions 2c, 2c+1).
            sq64 = small.tile([64, 4], F32)
            nc.sync.dma_start(out=sq64, in_=part[:, :])
            stt = small.tile([64, 2], F32)  # (S_tot, Q_tot)
            nc.vector.tensor_add(out=stt, in0=sq64[:, 0:2], in1=sq64[:, 2:4])

            # mean/inv on 64 partitions, packed in one [64, 2] tile.
            mi64 = small.tile([64, 2], F32)
            nc.vector.tensor_scalar_mul(out=mi64[:, 0:1], in0=stt[:, 0:1],
                                        scalar1=1.0 / NTOT)
            ex2t = small.tile([64, 1], F32)
            nc.vector.tensor_scalar_mul(out=ex2t, in0=stt[:, 1:2],
                                        scalar1=1.0 / NTOT)
            msq2 = small.tile([64, 1], F32)
            nc.vector.tensor_mul(out=msq2, in0=mi64[:, 0:1], in1=mi64[:, 0:1])
            var = small.tile([64, 1], F32)
            nc.vector.tensor_sub(out=var, in0=ex2t, in1=msq2)
            sbuf_eps = small.tile([64, 1], F32)
            nc.vector.memset(sbuf_eps, EPS)
            sd = small.tile([64, 1], F32)
            nc.scalar.activation(out=sd, in_=var,
                                 func=mybir.ActivationFunctionType.Sqrt,
                                 bias=sbuf_eps[:, :])
            nc.vector.reciprocal(out=mi64[:, 1:2], in_=sd)

            # Duplicate (mean, inv) to both h0 partitions of each channel:
            # partition p = c*2 + h0 reads row c of mi64.
            mi64_ap = mi64[:, :]
            mi_dup = bass.AP(
                tensor=mi64_ap.tensor, offset=mi64_ap.offset,
                ap=[mi64_ap.ap[0], [0, 2], mi64_ap.ap[1]],
            )
            mi = small.tile([128, 2], F32)
            nc.sync.dma_start(out=mi, in_=mi_dup)

            # A = gamma*inv ; Bc = beta - mean*A.
            a_sb = consts.tile([128, NL], F32)
            nc.vector.tensor_scalar_mul(out=a_sb, in0=g_sb,
                                        scalar1=mi[:, 1:2])
            ma = consts.tile([128, NL], F32)
            nc.vector.tensor_scalar_mul(out=ma, in0=a_sb, scalar1=mi[:, 0:1])
            bc_sb = consts.tile([128, NL], F32)
            nc.vector.tensor_sub(out=bc_sb, in0=b_sb, in1=ma)

            # Main loop: fused multiply-add + 1 MiB store per (branch, group).
            for j in range(NL):
                for g in range(B // WG):
                    o = outs.tile([128, WG * FB], F32)
                    xg = x_sb[:, g * WG:(g + 1) * WG, :].rearrange(
                        "p b f -> p (b f)")
                    nc.vector.tensor_scalar(
                        out=o, in0=xg,
                        scalar1=a_sb[:, j:j + 1], scalar2=bc_sb[:, j:j + 1],
                        op0=mybir.AluOpType.mult, op1=mybir.AluOpType.add,
                    )
                    nc.sync.dma_start(
                        out=out_re[j][:, g * WG:(g + 1) * WG, :], in_=o)
    # Run Bacc's compile pipeline (event-sem legalization, register
    # allocation); the PJRT execute path serializes without finalizing.
    nc.finalize()
    return nc


def _get_nc():
    if "nc" not in _NC_CACHE:
        _NC_CACHE["nc"] = _build()
    return _NC_CACHE["nc"]


def _run(inputs, **kwargs):
    x = np.ascontiguousarray(np.asarray(inputs["x"], dtype=np.float32))
    gamma = np.asarray(inputs["gamma"], dtype=np.float32)
    beta = np.asarray(inputs["beta"], dtype=np.float32)
    g128 = np.ascontiguousarray(np.repeat(gamma.T, 2, axis=0))  # [128, 16]
    b128 = np.ascontiguousarray(np.repeat(beta.T, 2, axis=0))
    in_maps = [
        {"x": x,
         "gn": np.ascontiguousarray(g128[:, i * NL:(i + 1) * NL]),
         "bn": np.ascontiguousarray(b128[:, i * NL:(i + 1) * NL])}
        for i in range(NCORES)
    ]
    nc = _get_nc()
    res = run_bass_kernel_spmd(nc, in_maps, core_ids=list(range(NCORES)), **kwargs)
    # Core i computed branches [i*NL, (i+1)*NL) -> channel block of NL*C.
    full = np.concatenate([r["out"] for r in res.results], axis=1)
    return full, res


def kernel(**inputs):
    full, _ = _run(inputs)
    return full


# revision 11
# speedup vs baseline: 1.5405x; 1.0194x over previous
"""Multi-branch BatchNorm2d (16 branches sharing one batch-stat reduction).

Computation (reference):
    mean/var over (B,H,W) per channel of x[32,64,32,32], then for each of
    N=16 branches: out[:, n*64:(n+1)*64] = gamma[n,c]*xhat + beta[n,c],
    giving out[32, 1024, 32, 32].

Strategy (8 NeuronCores, branch-parallel, no collectives):
  - x is replicated: every core reads the full 8 MiB x and computes the
    (B,H,W) mean/var locally. A 1 KB all-reduce would instead allow a
    batch-sharded read (1 MiB/core), but the ncfw collective measures
    70-80 us/call on this setup - far more than the 20 us of extra read.
    With no cross-core dependency, each core's span is independent of
    dispatch stagger.
  - SBUF layout [128, 32, 512]: partition p = c*2 + h0 (h0 = H half),
    free (b, (h1, w)). x is loaded in decreasing batch chunks so the
    per-batch bn_stats pipeline drains right behind the last DMA.
  - The (c,0)/(c,1) partition pair is summed via two small SBUF-to-SBUF
    DMAs (no HBM bounce), then mean = S/32768, inv = rsqrt(var+eps) are
    folded with gamma/beta into per-(branch,channel) scale A = gamma*inv,
    bias B = beta - mean*A.
  - Each core computes N/8 = 2 branches: 16 fused tensor_scalar ops
    (out = x*A + B) + 16 x 1 MiB DMA stores = 16 MiB of output writes per
    core, the HBM roofline for this memory-bound problem.
"""

import numpy as np

import concourse.bacc as bacc
import concourse.bass as bass
import concourse.tile as tile
from concourse import mybir
from concourse.bass_utils import run_bass_kernel_spmd

B, C, H, W = 32, 64, 32, 32
N = 16
NCORES = 8
NL = N // NCORES           # 2 branches per core
H2 = H // 2                # 16
FB = H2 * W                # 512 free elems per batch per partition
NTOT = float(B * H * W)    # 32768 elements reduced per channel
EPS = 1e-5
F32 = mybir.dt.float32

# Load chunks (in batches): big first so the read stream saturates early,
# small last so the stats tail behind the final DMA is short.
CHUNKS = [6, 6, 5, 5, 4, 3, 2, 1]
assert sum(CHUNKS) == B

WG = 4                     # batches per write group

_NC_CACHE = {}


def _build():
    # Bacc (not raw Bass): its generate_event_semaphores pass legalizes
    # instructions down to <=1 sync-wait each (walrus TS encodings cannot
    # carry more).
    nc = bacc.Bacc("TRN2", num_devices=NCORES, target_bir_lowering=False,
                   debug=False)
    x = nc.dram_tensor("x", [B, C, H, W], F32, kind="ExternalInput")
    gn = nc.dram_tensor("gn", [2 * C, NL], F32, kind="ExternalInput")
    bn = nc.dram_tensor("bn", [2 * C, NL], F32, kind="ExternalInput")
    out = nc.dram_tensor("out", [B, NL * C, H, W], F32, kind="ExternalOutput")

    # [128, 32, 512]: partition (c h0), free (b, h1*w)
    x_re = x.ap().rearrange("b c (h0 h1) w -> (c h0) b (h1 w)", h0=2)
    # [2, 128, 32, 512]
    out_re = out.ap().rearrange("b (n c) (h0 h1) w -> n (c h0) b (h1 w)",
                                n=NL, h0=2)

    with tile.TileContext(nc) as tc:
        with (
            tc.tile_pool(name="xin", bufs=1) as xin,
            tc.tile_pool(name="consts", bufs=1) as consts,
            tc.tile_pool(name="small", bufs=1) as small,
            tc.tile_pool(name="outs", bufs=6) as outs,
        ):
            # Per-(c,h0) gamma/beta for this core's branches, pre-transposed
            # on host: [128, 2].
            g_sb = consts.tile([2 * C, NL], F32)
            b_sb = consts.tile([2 * C, NL], F32)
            nc.gpsimd.dma_start(out=g_sb, in_=gn.ap())
            nc.gpsimd.dma_start(out=b_sb, in_=bn.ap())

            # Full x, loaded in batch chunks. Per chunk, two accumulating
            # passes pipeline behind the DMA in parallel: DVE computes the
            # chunk sum (tensor_scalar copy + accum_out, 2x perf mode), ACT
            # computes the chunk sum-of-squares (Square + accum_out).
            nchunk = len(CHUNKS)
            x_sb = xin.tile([2 * C, B, FB], F32)
            junk_s = small.tile([128, max(CHUNKS) * FB], F32, tag="junk_s")
            junk_q = small.tile([128, max(CHUNKS) * FB], F32, tag="junk_q")
            s_cols = small.tile([128, nchunk], F32)
            q_cols = small.tile([128, nchunk], F32)
            b0 = 0
            for ci, nb in enumerate(CHUNKS):
                nc.sync.dma_start(out=x_sb[:, b0:b0 + nb, :],
                                  in_=x_re[:, b0:b0 + nb, :])
                xc = x_sb[:, b0:b0 + nb, :].rearrange("p b f -> p (b f)")
                nc.vector.tensor_scalar(
                    out=junk_s[:, 0:nb * FB], in0=xc,
                    scalar1=1.0, scalar2=0.0, op0=mybir.AluOpType.mult,
                    op1=mybir.AluOpType.add,
                    accum_out=s_cols[:, ci:ci + 1])
                nc.scalar.activation(
                    out=junk_q[:, 0:nb * FB], in_=xc,
                    func=mybir.ActivationFunctionType.Square,
                    accum_out=q_cols[:, ci:ci + 1])
                b0 += nb

            # (S, Q) per partition (per H-half), then pair-combine via the
            # DVE 32-way partition permute: swapped[p] = part[p^1].
            part = small.tile([128, 2], F32)
            nc.vector.reduce_sum(out=part[:, 0:1], in_=s_cols,
                                 axis=mybir.AxisListType.X)
            nc.vector.reduce_sum(out=part[:, 1:2], in_=q_cols,
                                 axis=mybir.AxisListType.X)
            swapped = small.tile([128, 2], F32)
            pairswap = [i ^ 1 for i in range(32)]
            nc.vector.stream_shuffle(out=swapped, in_=part[:, :],
                                     mask=pairswap)
            stt = small.tile([128, 2], F32)  # (S_tot, Q_tot) per channel
            nc.vector.tensor_add(out=stt, in0=part[:, :], in1=swapped)

            mean = small.tile([128, 1], F32)
            nc.vector.tensor_scalar_mul(out=mean, in0=stt[:, 0:1],
                                        scalar1=1.0 / NTOT)
            ex2t = small.tile([128, 1], F32)
            nc.vector.tensor_scalar_mul(out=ex2t, in0=stt[:, 1:2],
                                        scalar1=1.0 / NTOT)
            msq2 = small.tile([128, 1], F32)
            nc.vector.tensor_mul(out=msq2, in0=mean, in1=mean)
            var = small.tile([128, 1], F32)
            nc.vector.tensor_sub(out=var, in0=ex2t, in1=msq2)
            sbuf_eps = small.tile([128, 1], F32)
            nc.vector.memset(sbuf_eps, EPS)
            sd = small.tile([128, 1], F32)
            nc.scalar.activation(out=sd, in_=var,
                                 func=mybir.ActivationFunctionType.Sqrt,
                                 bias=sbuf_eps[:, :])
            inv = small.tile([128, 1], F32)
            nc.vector.reciprocal(out=inv, in_=sd)

            # A = gamma*inv ; Bc = beta - mean*A.
            a_sb = consts.tile([128, NL], F32)
            nc.vector.tensor_scalar_mul(out=a_sb, in0=g_sb, scalar1=inv)
            ma = consts.tile([128, NL], F32)
            nc.vector.tensor_scalar_mul(out=ma, in0=a_sb, scalar1=mean)
            bc_sb = consts.tile([128, NL], F32)
            nc.vector.tensor_sub(out=bc_sb, in0=b_sb, in1=ma)

            # Main loop: fused multiply-add + 1 MiB store per (branch, group).
            for j in range(NL):
                for g in range(B // WG):
                    o = outs.tile([128, WG * FB], F32)
                    xg = x_sb[:, g * WG:(g + 1) * WG, :].rearrange(
                        "p b f -> p (b f)")
                    nc.vector.tensor_scalar(
                        out=o, in0=xg,
                        scalar1=a_sb[:, j:j + 1], scalar2=bc_sb[:, j:j + 1],
                        op0=mybir.AluOpType.mult, op1=mybir.AluOpType.add,
                    )
                    nc.sync.dma_start(
                        out=out_re[j][:, g * WG:(g + 1) * WG, :], in_=o)
    # Run Bacc's compile pipeline (event-sem legalization, register
    # allocation); the PJRT execute path serializes without finalizing.
    nc.finalize()
    return nc


def _get_nc():
    if "nc" not in _NC_CACHE:
        _NC_CACHE["nc"] = _build()
    return _NC_CACHE["nc"]


def _run(inputs, **kwargs):
    x = np.ascontiguousarray(np.asarray(inputs["x"], dtype=np.float32))
    gamma = np.asarray(inputs["gamma"], dtype=np.float32)
    beta = np.asarray(inputs["beta"], dtype=np.float32)
    g128 = np.ascontiguousarray(np.repeat(gamma.T, 2, axis=0))  # [128, 16]
    b128 = np.ascontiguousarray(np.repeat(beta.T, 2, axis=0))
    in_maps = [
        {"x": x,
         "gn": np.ascontiguousarray(g128[:, i * NL:(i + 1) * NL]),
         "bn": np.ascontiguousarray(b128[:, i * NL:(i + 1) * NL])}
        for i in range(NCORES)
    ]
    nc = _get_nc()
    res = run_bass_kernel_spmd(nc, in_maps, core_ids=list(range(NCORES)), **kwargs)
    # Core i computed branches [i*NL, (i+1)*NL) -> channel block of NL*C.
    full = np.concatenate([r["out"] for r in res.results], axis=1)
    return full, res


def kernel(**inputs):
    full, _ = _run(inputs)
    return full


# revision 18
# speedup vs baseline: 1.5740x; 1.0217x over previous
"""Multi-branch BatchNorm2d (16 branches sharing one batch-stat reduction).

Computation (reference):
    mean/var over (B,H,W) per channel of x[32,64,32,32], then for each of
    N=16 branches: out[:, n*64:(n+1)*64] = gamma[n,c]*xhat + beta[n,c],
    giving out[32, 1024, 32, 32].

Strategy (8 NeuronCores, branch-parallel, no collectives):
  - x is replicated: every core reads the full 8 MiB x and computes the
    (B,H,W) mean/var locally. A 1 KB all-reduce would instead allow a
    batch-sharded read (1 MiB/core), but the ncfw collective measures
    70-80 us/call on this setup - far more than the 20 us of extra read.
    With no cross-core dependency, each core's span is independent of
    dispatch stagger.
  - SBUF layout [128, 32, 512]: partition p = c*2 + h0 (h0 = H half),
    free (b, (h1, w)). x is loaded in decreasing batch chunks so the
    per-batch bn_stats pipeline drains right behind the last DMA.
  - The (c,0)/(c,1) partition pair is summed via two small SBUF-to-SBUF
    DMAs (no HBM bounce), then mean = S/32768, inv = rsqrt(var+eps) are
    folded with gamma/beta into per-(branch,channel) scale A = gamma*inv,
    bias B = beta - mean*A.
  - Each core computes N/8 = 2 branches: 16 fused tensor_scalar ops
    (out = x*A + B) + 16 x 1 MiB DMA stores = 16 MiB of output writes per
    core, the HBM roofline for this memory-bound problem.
"""

import numpy as np

import concourse.bacc as bacc
import concourse.bass as bass
import concourse.tile as tile
from concourse import mybir
from concourse.bass_utils import run_bass_kernel_spmd

B, C, H, W = 32, 64, 32, 32
N = 16
NCORES = 8
NL = N // NCORES           # 2 branches per core
H2 = H // 2                # 16
FB = H2 * W                # 512 free elems per batch per partition
NTOT = float(B * H * W)    # 32768 elements reduced per channel
EPS = 1e-5
F32 = mybir.dt.float32

# Load chunks (in batches): flat 1 MiB chunks keep the read stream at line
# rate; a small final chunk keeps the stats tail behind the last DMA short.
CHUNKS = [4, 4, 4, 4, 4, 4, 4, 3, 1]
assert sum(CHUNKS) == B

WG = 4                     # batches per write group

_NC_CACHE = {}


def _build():
    # Bacc (not raw Bass): its generate_event_semaphores pass legalizes
    # instructions down to <=1 sync-wait each (walrus TS encodings cannot
    # carry more).
    nc = bacc.Bacc("TRN2", num_devices=NCORES, target_bir_lowering=False,
                   debug=False)
    x = nc.dram_tensor("x", [B, C, H, W], F32, kind="ExternalInput")
    gn = nc.dram_tensor("gn", [2 * C, NL], F32, kind="ExternalInput")
    bn = nc.dram_tensor("bn", [2 * C, NL], F32, kind="ExternalInput")
    out = nc.dram_tensor("out", [B, NL * C, H, W], F32, kind="ExternalOutput")

    # [128, 32, 512]: partition (c h0), free (b, h1*w)
    x_re = x.ap().rearrange("b c (h0 h1) w -> (c h0) b (h1 w)", h0=2)
    # [2, 128, 32, 512]
    out_re = out.ap().rearrange("b (n c) (h0 h1) w -> n (c h0) b (h1 w)",
                                n=NL, h0=2)

    with tile.TileContext(nc) as tc:
        with (
            tc.tile_pool(name="xin", bufs=1) as xin,
            tc.tile_pool(name="consts", bufs=1) as consts,
            tc.tile_pool(name="small", bufs=1) as small,
            tc.tile_pool(name="outs", bufs=6) as outs,
        ):
            sbuf_eps = small.tile([128, 1], F32)
            nc.vector.memset(sbuf_eps, EPS)

            # Per-(c,h0) gamma/beta for this core's branches, pre-transposed
            # on host: [128, 2].
            g_sb = consts.tile([2 * C, NL], F32)
            b_sb = consts.tile([2 * C, NL], F32)
            nc.gpsimd.dma_start(out=g_sb, in_=gn.ap())
            nc.gpsimd.dma_start(out=b_sb, in_=bn.ap())

            # Full x, loaded in batch chunks. Per chunk, two accumulating
            # passes pipeline behind the DMA in parallel: ACT computes the
            # chunk sum (Copy + accum_out; Copy needs no LUT table), DVE the
            # chunk sum-of-squares (tensor_tensor_reduce x*x).
            nchunk = len(CHUNKS)
            x_sb = xin.tile([2 * C, B, FB], F32)
            junk_s = small.tile([128, max(CHUNKS) * FB], F32, tag="junk_s")
            junk_q = small.tile([128, max(CHUNKS) * FB], F32, tag="junk_q")
            s_cols = small.tile([128, nchunk], F32)
            q_cols = small.tile([128, nchunk], F32)
            b0 = 0
            for ci, nb in enumerate(CHUNKS):
                nc.sync.dma_start(out=x_sb[:, b0:b0 + nb, :],
                                  in_=x_re[:, b0:b0 + nb, :])
                xc = x_sb[:, b0:b0 + nb, :].rearrange("p b f -> p (b f)")
                nc.vector.tensor_scalar(
                    out=junk_s[:, 0:nb * FB], in0=xc,
                    scalar1=1.0, scalar2=0.0, op0=mybir.AluOpType.mult,
                    op1=mybir.AluOpType.add,
                    accum_out=s_cols[:, ci:ci + 1])
                nc.scalar.activation(
                    out=junk_q[:, 0:nb * FB], in_=xc,
                    func=mybir.ActivationFunctionType.Square,
                    accum_out=q_cols[:, ci:ci + 1])
                b0 += nb

            # (S, Q) per partition (per H-half), then pair-combine via the
            # DVE 32-way partition permute: swapped[p] = part[p^1].
            part = small.tile([128, 2], F32)
            nc.vector.reduce_sum(out=part[:, 0:1], in_=s_cols,
                                 axis=mybir.AxisListType.X)
            nc.vector.reduce_sum(out=part[:, 1:2], in_=q_cols,
                                 axis=mybir.AxisListType.X)
            swapped = small.tile([128, 2], F32)
            pairswap = [i ^ 1 for i in range(32)]
            nc.vector.stream_shuffle(out=swapped, in_=part[:, :],
                                     mask=pairswap)
            stt = small.tile([128, 2], F32)  # (S_tot, Q_tot) per channel
            nc.vector.tensor_add(out=stt, in0=part[:, :], in1=swapped)

            mean = small.tile([128, 1], F32)
            nc.vector.tensor_scalar_mul(out=mean, in0=stt[:, 0:1],
                                        scalar1=1.0 / NTOT)
            ex2t = small.tile([128, 1], F32)
            nc.vector.tensor_scalar_mul(out=ex2t, in0=stt[:, 1:2],
                                        scalar1=1.0 / NTOT)
            msq2 = small.tile([128, 1], F32)
            nc.vector.tensor_mul(out=msq2, in0=mean, in1=mean)
            var = small.tile([128, 1], F32)
            nc.vector.tensor_sub(out=var, in0=ex2t, in1=msq2)
            sd = small.tile([128, 1], F32)
            nc.scalar.activation(out=sd, in_=var,
                                 func=mybir.ActivationFunctionType.Sqrt,
                                 bias=sbuf_eps[:, :])
            inv = small.tile([128, 1], F32)
            nc.vector.reciprocal(out=inv, in_=sd)

            # A = gamma*inv ; Bc = beta - mean*A.
            a_sb = consts.tile([128, NL], F32)
            nc.vector.tensor_scalar_mul(out=a_sb, in0=g_sb, scalar1=inv)
            ma = consts.tile([128, NL], F32)
            nc.vector.tensor_scalar_mul(out=ma, in0=a_sb, scalar1=mean)
            bc_sb = consts.tile([128, NL], F32)
            nc.vector.tensor_sub(out=bc_sb, in0=b_sb, in1=ma)

            # Main loop: fused multiply-add + 1 MiB store per (branch, group).
            for j in range(NL):
                for g in range(B // WG):
                    o = outs.tile([128, WG * FB], F32)
                    xg = x_sb[:, g * WG:(g + 1) * WG, :].rearrange(
                        "p b f -> p (b f)")
                    nc.vector.tensor_scalar(
                        out=o, in0=xg,
                        scalar1=a_sb[:, j:j + 1], scalar2=bc_sb[:, j:j + 1],
                        op0=mybir.AluOpType.mult, op1=mybir.AluOpType.add,
                    )
                    nc.sync.dma_start(
                        out=out_re[j][:, g * WG:(g + 1) * WG, :], in_=o)
    # Run Bacc's compile pipeline (event-sem legalization, register
    # allocation); the PJRT execute path serializes without finalizing.
    nc.finalize()
    return nc


def _get_nc():
    if "nc" not in _NC_CACHE:
        _NC_CACHE["nc"] = _build()
    return _NC_CACHE["nc"]


def _run(inputs, **kwargs):
    x = np.ascontiguousarray(np.asarray(inputs["x"], dtype=np.float32))
    gamma = np.asarray(inputs["gamma"], dtype=np.float32)
    beta = np.asarray(inputs["beta"], dtype=np.float32)
    g128 = np.ascontiguousarray(np.repeat(gamma.T, 2, axis=0))  # [128, 16]
    b128 = np.ascontiguousarray(np.repeat(beta.T, 2, axis=0))
    in_maps = [
        {"x": x,
         "gn": np.ascontiguousarray(g128[:, i * NL:(i + 1) * NL]),
         "bn": np.ascontiguousarray(b128[:, i * NL:(i + 1) * NL])}
        for i in range(NCORES)
    ]
    nc = _get_nc()
    res = run_bass_kernel_spmd(nc, in_maps, core_ids=list(range(NCORES)), **kwargs)
    # Core i computed branches [i*NL, (i+1)*NL) -> channel block of NL*C.
    full = np.concatenate([r["out"] for r in res.results], axis=1)
    return full, res


def kernel(**inputs):
    full, _ = _run(inputs)
    return full


# revision 19
# speedup vs baseline: 1.5792x; 1.0033x over previous
"""Multi-branch BatchNorm2d (16 branches sharing one batch-stat reduction).

Computation (reference):
    mean/var over (B,H,W) per channel of x[32,64,32,32], then for each of
    N=16 branches: out[:, n*64:(n+1)*64] = gamma[n,c]*xhat + beta[n,c],
    giving out[32, 1024, 32, 32].

Strategy (8 NeuronCores, branch-parallel, no collectives):
  - x is replicated: every core reads the full 8 MiB x and computes the
    (B,H,W) mean/var locally. A 1 KB all-reduce would instead allow a
    batch-sharded read (1 MiB/core), but the ncfw collective measures
    70-80 us/call on this setup - far more than the 20 us of extra read.
    With no cross-core dependency, each core's span is independent of
    dispatch stagger.
  - SBUF layout [128, 32, 512]: partition p = c*2 + h0 (h0 = H half),
    free (b, (h1, w)). x is loaded in decreasing batch chunks so the
    per-batch bn_stats pipeline drains right behind the last DMA.
  - The (c,0)/(c,1) partition pair is summed via two small SBUF-to-SBUF
    DMAs (no HBM bounce), then mean = S/32768, inv = rsqrt(var+eps) are
    folded with gamma/beta into per-(branch,channel) scale A = gamma*inv,
    bias B = beta - mean*A.
  - Each core computes N/8 = 2 branches: 16 fused tensor_scalar ops
    (out = x*A + B) + 16 x 1 MiB DMA stores = 16 MiB of output writes per
    core, the HBM roofline for this memory-bound problem.
"""

import numpy as np

import concourse.bacc as bacc
import concourse.bass as bass
import concourse.tile as tile
from concourse import mybir
from concourse.bass_utils import run_bass_kernel_spmd

B, C, H, W = 32, 64, 32, 32
N = 16
NCORES = 8
NL = N // NCORES           # 2 branches per core
H2 = H // 2                # 16
FB = H2 * W                # 512 free elems per batch per partition
NTOT = float(B * H * W)    # 32768 elements reduced per channel
EPS = 1e-5
F32 = mybir.dt.float32

# Load chunks (in batches): flat 1 MiB chunks keep the read stream at line
# rate; a small final chunk keeps the stats tail behind the last DMA short.
CHUNKS = [4, 4, 4, 4, 4, 4, 4, 3, 1]
assert sum(CHUNKS) == B

WG = 4                     # batches per write group

_NC_CACHE = {}


def _build():
    # Bacc (not raw Bass): its generate_event_semaphores pass legalizes
    # instructions down to <=1 sync-wait each (walrus TS encodings cannot
    # carry more).
    nc = bacc.Bacc("TRN2", num_devices=NCORES, target_bir_lowering=False,
                   debug=False)
    x = nc.dram_tensor("x", [B, C, H, W], F32, kind="ExternalInput")
    gn = nc.dram_tensor("gn", [2 * C, NL], F32, kind="ExternalInput")
    bn = nc.dram_tensor("bn", [2 * C, NL], F32, kind="ExternalInput")
    out = nc.dram_tensor("out", [B, NL * C, H, W], F32, kind="ExternalOutput")

    # [128, 32, 512]: partition (c h0), free (b, h1*w)
    x_re = x.ap().rearrange("b c (h0 h1) w -> (c h0) b (h1 w)", h0=2)
    # [2, 128, 32, 512]
    out_re = out.ap().rearrange("b (n c) (h0 h1) w -> n (c h0) b (h1 w)",
                                n=NL, h0=2)

    with tile.TileContext(nc) as tc:
        with (
            tc.tile_pool(name="xin", bufs=1) as xin,
            tc.tile_pool(name="consts", bufs=1) as consts,
            tc.tile_pool(name="small", bufs=1) as small,
            tc.tile_pool(name="outs", bufs=10) as outs,
        ):
            sbuf_eps = small.tile([128, 1], F32)
            nc.vector.memset(sbuf_eps, EPS)

            # Per-(c,h0) gamma/beta for this core's branches, pre-transposed
            # on host: [128, 2].
            g_sb = consts.tile([2 * C, NL], F32)
            b_sb = consts.tile([2 * C, NL], F32)
            nc.gpsimd.dma_start(out=g_sb, in_=gn.ap())
            nc.gpsimd.dma_start(out=b_sb, in_=bn.ap())

            # Full x, loaded in batch chunks. Per chunk, two accumulating
            # passes pipeline behind the DMA in parallel: ACT computes the
            # chunk sum (Copy + accum_out; Copy needs no LUT table), DVE the
            # chunk sum-of-squares (tensor_tensor_reduce x*x).
            nchunk = len(CHUNKS)
            x_sb = xin.tile([2 * C, B, FB], F32)
            junk_s = small.tile([128, max(CHUNKS) * FB], F32, tag="junk_s")
            junk_q = small.tile([128, max(CHUNKS) * FB], F32, tag="junk_q")
            s_cols = small.tile([128, nchunk], F32)
            q_cols = small.tile([128, nchunk], F32)
            b0 = 0
            for ci, nb in enumerate(CHUNKS):
                nc.sync.dma_start(out=x_sb[:, b0:b0 + nb, :],
                                  in_=x_re[:, b0:b0 + nb, :])
                xc = x_sb[:, b0:b0 + nb, :].rearrange("p b f -> p (b f)")
                nc.vector.tensor_scalar(
                    out=junk_s[:, 0:nb * FB], in0=xc,
                    scalar1=1.0, scalar2=0.0, op0=mybir.AluOpType.mult,
                    op1=mybir.AluOpType.add,
                    accum_out=s_cols[:, ci:ci + 1])
                nc.scalar.activation(
                    out=junk_q[:, 0:nb * FB], in_=xc,
                    func=mybir.ActivationFunctionType.Square,
                    accum_out=q_cols[:, ci:ci + 1])
                b0 += nb

            # (S, Q) per partition (per H-half), then pair-combine via the
            # DVE 32-way partition permute: swapped[p] = part[p^1].
            part = small.tile([128, 2], F32)
            nc.vector.reduce_sum(out=part[:, 0:1], in_=s_cols,
                                 axis=mybir.AxisListType.X)
            nc.vector.reduce_sum(out=part[:, 1:2], in_=q_cols,
                                 axis=mybir.AxisListType.X)
            swapped = small.tile([128, 2], F32)
            pairswap = [i ^ 1 for i in range(32)]
            nc.vector.stream_shuffle(out=swapped, in_=part[:, :],
                                     mask=pairswap)
            stt = small.tile([128, 2], F32)  # (S_tot, Q_tot) per channel
            nc.vector.tensor_add(out=stt, in0=part[:, :], in1=swapped)

            mean = small.tile([128, 1], F32)
            nc.vector.tensor_scalar_mul(out=mean, in0=stt[:, 0:1],
                                        scalar1=1.0 / NTOT)
            ex2t = small.tile([128, 1], F32)
            nc.vector.tensor_scalar_mul(out=ex2t, in0=stt[:, 1:2],
                                        scalar1=1.0 / NTOT)
            msq2 = small.tile([128, 1], F32)
            nc.vector.tensor_mul(out=msq2, in0=mean, in1=mean)
            var = small.tile([128, 1], F32)
            nc.vector.tensor_sub(out=var, in0=ex2t, in1=msq2)
            sd = small.tile([128, 1], F32)
            nc.scalar.activation(out=sd, in_=var,
                                 func=mybir.ActivationFunctionType.Sqrt,
                                 bias=sbuf_eps[:, :])
            inv = small.tile([128, 1], F32)
            nc.vector.reciprocal(out=inv, in_=sd)

            # A = gamma*inv ; Bc = beta - mean*A.
            a_sb = consts.tile([128, NL], F32)
            nc.vector.tensor_scalar_mul(out=a_sb, in0=g_sb, scalar1=inv)
            ma = consts.tile([128, NL], F32)
            nc.vector.tensor_scalar_mul(out=ma, in0=a_sb, scalar1=mean)
            bc_sb = consts.tile([128, NL], F32)
            nc.vector.tensor_sub(out=bc_sb, in0=b_sb, in1=ma)

            # Main loop: fused multiply-add + 1 MiB store per (branch, group).
            for j in range(NL):
                for g in range(B // WG):
                    o = outs.tile([128, WG * FB], F32)
                    xg = x_sb[:, g * WG:(g + 1) * WG, :].rearrange(
                        "p b f -> p (b f)")
                    nc.vector.tensor_scalar(
                        out=o, in0=xg,
                        scalar1=a_sb[:, j:j + 1], scalar2=bc_sb[:, j:j + 1],
                        op0=mybir.AluOpType.mult, op1=mybir.AluOpType.add,
                    )
                    nc.sync.dma_start(
                        out=out_re[j][:, g * WG:(g + 1) * WG, :], in_=o)
    # Run Bacc's compile pipeline (event-sem legalization, register
    # allocation); the PJRT execute path serializes without finalizing.
    nc.finalize()
    return nc


def _get_nc():
    if "nc" not in _NC_CACHE:
        _NC_CACHE["nc"] = _build()
    return _NC_CACHE["nc"]


def _run(inputs, **kwargs):
    x = np.ascontiguousarray(np.asarray(inputs["x"], dtype=np.float32))
    gamma = np.asarray(inputs["gamma"], dtype=np.float32)
    beta = np.asarray(inputs["beta"], dtype=np.float32)
    g128 = np.ascontiguousarray(np.repeat(gamma.T, 2, axis=0))  # [128, 16]
    b128 = np.ascontiguousarray(np.repeat(beta.T, 2, axis=0))
    in_maps = [
        {"x": x,
         "gn": np.ascontiguousarray(g128[:, i * NL:(i + 1) * NL]),
         "bn": np.ascontiguousarray(b128[:, i * NL:(i + 1) * NL])}
        for i in range(NCORES)
    ]
    nc = _get_nc()
    res = run_bass_kernel_spmd(nc, in_maps, core_ids=list(range(NCORES)), **kwargs)
    # Core i computed branches [i*NL, (i+1)*NL) -> channel block of NL*C.
    full = np.concatenate([r["out"] for r in res.results], axis=1)
    return full, res


def kernel(**inputs):
    full, _ = _run(inputs)
    return full
